# revision 8
# baseline (speedup 1.0000x reference)
"""Cross-attention kernel for 8 TRN2 NeuronCores (Bass/Tile, SPMD).

Problem: B=4, SQ=SKV=2048, D_MODEL=1024, H=16 heads, Dh=64, fp32.
    Q = q @ Wq.T + bq; K = kv @ Wk.T + bk; V = kv @ Wv.T + bv
    out = softmax(Q K^T / sqrt(Dh)) V  -> concat heads -> @ Wo.T + bo

Sharding: 8 cores = 4 batches x 2 head-groups (8 heads each). Each core
computes its batch's projections for its 8 heads, full attention for those
heads, and a partial out-projection (its 512 columns of the head-concat dim).
The host sums the two partials per batch (no device collectives needed).

Device layout (everything transposed so matmul contractions sit on the
partition axis):
  - inputs qT/kvT: (1024, 2048) = x[b].T
  - QT, KT: (512, 2048) = heads-major (8*64 rows), computed as W.T-chunks
    (stationary) x xT (moving)
  - Vhat: (2048, 520) = per head [V_h (64 cols) | 1.0], the ones column comes
    from a zero weight column + bias 1.0; it makes the PV matmul emit the
    softmax denominator as row 64 of each head's output
  - scores^T tiles (s=128, q=512x2): K=64 matmuls; exp on ACT with scale=1/8
    (no max subtraction: scores ~ N(0,1), exp is fp32-safe)
  - P^T V-hat: PSUM-accumulated over 16 s-blocks -> (65, 1024) per (head, jc)
  - normalize: reciprocal of row 64, partition-broadcast via DRAM bounce,
    multiply rows 0..63 -> attnT (512, 2048)
  - out-projection: attnT chunks stationary x woT moving -> out (2048, 1024)
    partial, bias added on head-group-0 cores only.
"""

import numpy as np

B = 4
S = 2048          # SQ == SKV
D = 1024
H_PER_CORE = 8
DH = 64
DC = H_PER_CORE * DH            # 512 head-concat dims per core
DHP = DH + 1                    # V-hat column block per head (64 + ones col)
N_CORES = 8
FP32 = None                     # set at build time (mybir.dt.float32)

_CACHE = {}


def _build_program():
    import concourse.bass as bass
    import concourse.tile as tile
    from concourse import bacc, mybir

    f32 = mybir.dt.float32
    nc = bacc.Bacc("TRN2", target_bir_lowering=False, debug=False,
                   enable_asserts=False, num_devices=N_CORES)

    qT = nc.dram_tensor("qT", [D, S], f32, kind="ExternalInput").ap()
    kvT = nc.dram_tensor("kvT", [D, S], f32, kind="ExternalInput").ap()
    wqT = nc.dram_tensor("wqT", [D, DC], f32, kind="ExternalInput").ap()
    wkT = nc.dram_tensor("wkT", [D, DC], f32, kind="ExternalInput").ap()
    wvh = nc.dram_tensor("wvh", [D, H_PER_CORE * DHP], f32, kind="ExternalInput").ap()
    bq = nc.dram_tensor("bq", [DC], f32, kind="ExternalInput").ap()
    bk = nc.dram_tensor("bk", [DC], f32, kind="ExternalInput").ap()
    bvh = nc.dram_tensor("bvh", [H_PER_CORE * DHP], f32, kind="ExternalInput").ap()
    woT = nc.dram_tensor("woT", [DC, D], f32, kind="ExternalInput").ap()
    bo = nc.dram_tensor("bo", [D], f32, kind="ExternalInput").ap()
    out = nc.dram_tensor("out", [S, D], f32, kind="ExternalOutput").ap()

    VW = H_PER_CORE * DHP       # 520
    KC = D // 128               # 8 contraction chunks for projections
    NM = DC // 128              # 4 partition chunks of QT/KT

    with tile.TileContext(nc) as tc:
        # ---- persistent SBUF tensors --------------------------------------
        with tc.tile_pool(name="persist", bufs=1) as persist:
            qt_t = [persist.tile([128, S], f32, tag=f"qt{m}", name=f"qt{m}") for m in range(NM)]
            kt_t = [persist.tile([128, S], f32, tag=f"kt{m}", name=f"kt{m}") for m in range(NM)]
            vh_t = [persist.tile([128, VW], f32, tag=f"vh{sb}", name=f"vh{sb}") for sb in range(S // 128)]

            # biases: bq/bk as (128, NM) per-partition scalars; bvh broadcast
            bq_t = persist.tile([128, NM], f32, tag="bq")
            bk_t = persist.tile([128, NM], f32, tag="bk")
            bvh_t = persist.tile([128, VW], f32, tag="bvh")
            bo_t = persist.tile([128, D], f32, tag="bo")

            def col_ap(vec, n):  # (n*128,) dram vector -> (128, n) column tile ap
                return bass.AP(tensor=vec.tensor, offset=vec.offset,
                               ap=[[1, 128], [128, n]])

            def bcast_ap(vec, p, w):  # (w,) dram vector -> (p, w) broadcast
                return bass.AP(tensor=vec.tensor, offset=vec.offset,
                               ap=[[0, p], [1, w]])

            nc.sync.dma_start(out=bq_t, in_=col_ap(bq, NM))
            nc.sync.dma_start(out=bk_t, in_=col_ap(bk, NM))
            nc.sync.dma_start(out=bvh_t, in_=bcast_ap(bvh, 128, VW))
            nc.sync.dma_start(out=bo_t, in_=bcast_ap(bo, 128, D))

            # ---- phase 1: projections, in 4 passes over s-quarters --------
            SQW = 512                       # s-quarter width
            with tc.tile_pool(name="wpool", bufs=1) as wpool:
                wq_t = [wpool.tile([128, DC], f32, tag=f"wq{k}", name=f"wq{k}") for k in range(KC)]
                wk_t = [wpool.tile([128, DC], f32, tag=f"wk{k}", name=f"wk{k}") for k in range(KC)]
                wv_t = [wpool.tile([128, VW], f32, tag=f"wv{k}", name=f"wv{k}") for k in range(KC)]
                for k in range(KC):
                    nc.sync.dma_start(out=wq_t[k], in_=wqT[k * 128:(k + 1) * 128, :])
                    nc.sync.dma_start(out=wk_t[k], in_=wkT[k * 128:(k + 1) * 128, :])
                    nc.sync.dma_start(out=wv_t[k], in_=wvh[k * 128:(k + 1) * 128, :])

                with tc.tile_pool(name="xq", bufs=1) as xq, \
                     tc.tile_pool(name="xkv", bufs=1) as xkv, \
                     tc.tile_pool(name="pp", bufs=2, space="PSUM") as pp, \
                     tc.tile_pool(name="ppv", bufs=2, space="PSUM") as ppv:
                    for sq in range(S // SQW):
                        ssl = slice(sq * SQW, (sq + 1) * SQW)
                        q_c = [xq.tile([128, SQW], f32, tag=f"q{k}", name=f"q{k}") for k in range(KC)]
                        kv_c = [xkv.tile([128, SQW], f32, tag=f"kv{k}", name=f"kv{k}") for k in range(KC)]
                        for k in range(KC):
                            nc.sync.dma_start(out=q_c[k], in_=qT[k * 128:(k + 1) * 128, ssl])
                            nc.sync.dma_start(out=kv_c[k], in_=kvT[k * 128:(k + 1) * 128, ssl])

                        for m in range(NM):
                            msl = slice(m * 128, (m + 1) * 128)
                            ps = pp.tile([128, SQW], f32, tag="proj")
                            for k in range(KC):
                                nc.tensor.matmul(ps, wq_t[k][:, msl], q_c[k],
                                                 start=(k == 0), stop=(k == KC - 1))
                            nc.vector.tensor_scalar_add(qt_t[m][:, ssl], ps, bq_t[:, m:m + 1])
                        for m in range(NM):
                            msl = slice(m * 128, (m + 1) * 128)
                            ps = pp.tile([128, SQW], f32, tag="proj")
                            for k in range(KC):
                                nc.tensor.matmul(ps, wk_t[k][:, msl], kv_c[k],
                                                 start=(k == 0), stop=(k == KC - 1))
                            nc.vector.tensor_scalar_add(kt_t[m][:, ssl], ps, bk_t[:, m:m + 1])
                        for sm in range(SQW // 128):
                            sb = sq * (SQW // 128) + sm
                            smsl = slice(sm * 128, (sm + 1) * 128)
                            psv = ppv.tile([128, 1024], f32, tag="vproj")
                            for k in range(KC):
                                nc.tensor.matmul(psv[:, 0:512], kv_c[k][:, smsl], wv_t[k][:, 0:512],
                                                 start=(k == 0), stop=(k == KC - 1))
                                nc.tensor.matmul(psv[:, 512:VW], kv_c[k][:, smsl], wv_t[k][:, 512:VW],
                                                 start=(k == 0), stop=(k == KC - 1))
                            nc.vector.tensor_add(vh_t[sb], psv[:, 0:VW], bvh_t)

            # ---- phase 2: attention per (head, q-chunk of 1024) -----------
            # attnT tiles live in their own pool so their SBUF space only
            # exists after the phase-1 weight pool is released
            attn_cm = tc.tile_pool(name="attn", bufs=1)
            attn_pool = attn_cm.__enter__()
            at_t = [attn_pool.tile([128, S], f32, tag=f"at{m}", name=f"at{m}")
                    for m in range(NM)]
            JW = 1024
            with tc.tile_pool(name="sps", bufs=2, space="PSUM") as sps, \
                 tc.tile_pool(name="pvs", bufs=2, space="PSUM") as pvs, \
                 tc.tile_pool(name="pt", bufs=3) as ptp, \
                 tc.tile_pool(name="nrm", bufs=3) as nrm, \
                 tc.tile_pool(name="dscr", bufs=3, space="DRAM") as dscr:
                for h in range(H_PER_CORE):
                    ht = h // 2
                    hsl = slice((h % 2) * 64, (h % 2) * 64 + 64)
                    vsl = slice(h * DHP, (h + 1) * DHP)
                    for jc in range(S // JW):
                        jsl = slice(jc * JW, (jc + 1) * JW)
                        pv = pvs.tile([DHP, JW], f32, tag="pv")
                        for sb in range(S // 128):
                            sbsl = slice(sb * 128, (sb + 1) * 128)
                            sp = sps.tile([128, JW], f32, tag="sc")
                            for n in range(JW // 512):
                                nc.tensor.matmul(
                                    sp[:, n * 512:(n + 1) * 512],
                                    kt_t[ht][hsl, sbsl],
                                    qt_t[ht][hsl, jc * JW + n * 512: jc * JW + (n + 1) * 512],
                                    start=True, stop=True)
                            p_t = ptp.tile([128, JW], f32, tag="p")
                            nc.scalar.activation(p_t, sp, mybir.ActivationFunctionType.Exp,
                                                 scale=0.125)
                            for n in range(JW // 512):
                                nsl = slice(n * 512, (n + 1) * 512)
                                nc.tensor.matmul(pv[:, nsl], vh_t[sb][:, vsl], p_t[:, nsl],
                                                 start=(sb == 0), stop=(sb == S // 128 - 1))
                        # normalize rows 0..63 by reciprocal of row 64
                        rec = nrm.tile([1, JW], f32, tag="rec")
                        nc.vector.reciprocal(rec, pv[64:65, :])
                        scr = dscr.tile([1, JW], f32, tag="scr")
                        nc.sync.dma_start(out=scr, in_=rec)
                        recb = nrm.tile([64, JW], f32, tag="recb")
                        sc = scr[0, :]
                        nc.sync.dma_start(
                            out=recb,
                            in_=bass.AP(tensor=sc.tensor, offset=sc.offset,
                                        ap=[[0, 64]] + sc.ap))
                        nc.vector.tensor_mul(at_t[ht][hsl, jsl], pv[0:64, :], recb)

            # ---- phase 3: partial out-projection --------------------------
            with tc.tile_pool(name="wo", bufs=1) as wop, \
                 tc.tile_pool(name="ops", bufs=2, space="PSUM") as ops, \
                 tc.tile_pool(name="ot", bufs=3) as otp:
                wo_t = [wop.tile([128, D], f32, tag=f"wo{k}", name=f"wo{k}") for k in range(NM)]
                for k in range(NM):
                    nc.sync.dma_start(out=wo_t[k], in_=woT[k * 128:(k + 1) * 128, :])
                for qm in range(S // 128):
                    qsl = slice(qm * 128, (qm + 1) * 128)
                    for n in range(D // 512):
                        nsl = slice(n * 512, (n + 1) * 512)
                        po = ops.tile([128, 512], f32, tag="po")
                        for k in range(NM):
                            nc.tensor.matmul(po, at_t[k][:, qsl], wo_t[k][:, nsl],
                                             start=(k == 0), stop=(k == NM - 1))
                        o_t = otp.tile([128, 512], f32, tag="o")
                        nc.vector.tensor_add(o_t, po, bo_t[:, nsl])
                        nc.sync.dma_start(out=out[qsl, nsl], in_=o_t)
            attn_cm.__exit__(None, None, None)

    nc.compile()
    return nc


def _get_runner():
    """Build the program once and return a cached jitted SPMD runner."""
    if "runner" in _CACHE:
        return _CACHE["runner"]

    import jax
    import jax.numpy as jnp
    from jax.sharding import Mesh, PartitionSpec
    from jax.experimental.shard_map import shard_map
    from concourse import mybir
    from concourse.bass2jax import (_bass_exec_p, install_neuronx_cc_hook,
                                    partition_id_tensor)

    nc = _build_program()
    install_neuronx_cc_hook()

    partition_name = nc.partition_id_tensor.name if nc.partition_id_tensor else None
    in_names, out_names, out_avals, zero_shapes = [], [], [], []
    for alloc in nc.m.functions[0].allocations:
        if not isinstance(alloc, mybir.MemoryLocationSet):
            continue
        name = alloc.memorylocations[0].name
        if alloc.kind == "ExternalInput":
            if name != partition_name:
                in_names.append(name)
        elif alloc.kind == "ExternalOutput":
            out_names.append(name)
            shape = tuple(alloc.tensor_shape)
            dtype = mybir.dt.np(alloc.dtype)
            out_avals.append(jax.core.ShapedArray(shape, dtype))
            zero_shapes.append((shape, dtype))
    n_params = len(in_names)
    n_outs = len(out_avals)
    all_in_names = list(in_names) + list(out_names)
    if partition_name is not None:
        all_in_names.append(partition_name)
    donate = tuple(range(n_params, n_params + n_outs))

    def _body(*args):
        operands = list(args)
        if partition_name is not None:
            operands.append(partition_id_tensor())
        outs = _bass_exec_p.bind(
            *operands,
            out_avals=tuple(out_avals),
            in_names=tuple(all_in_names),
            out_names=tuple(out_names),
            lowering_input_output_aliases=(),
            sim_require_finite=True,
            sim_require_nnan=True,
            nc=nc,
        )
        return tuple(outs)

    devices = jax.devices()[:N_CORES]
    mesh = Mesh(np.asarray(devices), ("core",))
    in_specs = (PartitionSpec("core"),) * (n_params + n_outs)
    out_specs = (PartitionSpec("core"),) * n_outs
    sharded = jax.jit(
        shard_map(_body, mesh=mesh, in_specs=in_specs, out_specs=out_specs,
                  check_rep=False),
        donate_argnums=donate, keep_unused=True)

    def run(in_maps):
        concat_in = [np.concatenate([np.asarray(m[name]) for m in in_maps], axis=0)
                     for name in in_names]
        concat_zeros = [np.zeros((N_CORES * s[0], *s[1:]), d) for s, d in zero_shapes]
        out_arrs = sharded(*concat_in, *concat_zeros)
        out_arrs = [np.asarray(a) for a in jax.block_until_ready(out_arrs)]
        return [
            {name: out_arrs[i].reshape(N_CORES, *out_avals[i].shape)[c]
             for i, name in enumerate(out_names)}
            for c in range(N_CORES)
        ]

    _CACHE["internals"] = {
        "sharded": sharded, "mesh": mesh, "in_names": in_names,
        "out_names": out_names, "zero_shapes": zero_shapes, "nc": nc,
    }
    _CACHE["runner"] = run
    return run


def _prep_in_maps(query, key_value, Wq, bq, Wk, bk, Wv, bv, Wo, bo):
    f = np.float32
    in_maps = []
    for c in range(N_CORES):
        b, hg = c // 2, c % 2
        sl = slice(hg * DC, (hg + 1) * DC)
        wv_s = np.asarray(Wv, f)[sl, :].T.reshape(D, H_PER_CORE, DH)
        wvh = np.concatenate([wv_s, np.zeros((D, H_PER_CORE, 1), f)], axis=2)
        bv_s = np.asarray(bv, f)[sl].reshape(H_PER_CORE, DH)
        bvh = np.concatenate([bv_s, np.ones((H_PER_CORE, 1), f)], axis=1)
        in_maps.append({
            "qT": np.ascontiguousarray(np.asarray(query, f)[b].T),
            "kvT": np.ascontiguousarray(np.asarray(key_value, f)[b].T),
            "wqT": np.ascontiguousarray(np.asarray(Wq, f)[sl, :].T),
            "wkT": np.ascontiguousarray(np.asarray(Wk, f)[sl, :].T),
            "wvh": np.ascontiguousarray(wvh.reshape(D, H_PER_CORE * DHP)),
            "bq": np.ascontiguousarray(np.asarray(bq, f)[sl]),
            "bk": np.ascontiguousarray(np.asarray(bk, f)[sl]),
            "bvh": np.ascontiguousarray(bvh.reshape(H_PER_CORE * DHP)),
            "woT": np.ascontiguousarray(np.asarray(Wo, f)[:, sl].T),
            "bo": (np.asarray(bo, f) if hg == 0 else np.zeros(D, f)),
        })
    return in_maps


def kernel(query, key_value, Wq, bq, Wk, bk, Wv, bv, Wo, bo):
    run = _get_runner()
    in_maps = _prep_in_maps(query, key_value, Wq, bq, Wk, bk, Wv, bv, Wo, bo)
    results = run(in_maps)
    out = np.empty((B, S, D), np.float32)
    for b in range(B):
        out[b] = results[2 * b]["out"] + results[2 * b + 1]["out"]
    return out


# revision 9
# speedup vs baseline: 32.1842x; 32.1842x over previous
"""Cross-attention kernel for 8 TRN2 NeuronCores (Bass/Tile, SPMD).

Problem: B=4, SQ=SKV=2048, D_MODEL=1024, H=16 heads, Dh=64, fp32.
    Q = q @ Wq.T + bq; K = kv @ Wk.T + bk; V = kv @ Wv.T + bv
    out = softmax(Q K^T / sqrt(Dh)) V  -> concat heads -> @ Wo.T + bo

Sharding: 8 cores = 4 batches x 2 head-groups (8 heads each). Each core
computes its batch's projections for its 8 heads, full attention for those
heads, and a partial out-projection (its 512 columns of the head-concat dim).
The host sums the two partials per batch (no device collectives needed).

Device layout (everything transposed so matmul contractions sit on the
partition axis):
  - inputs qT/kvT: (1024, 2048) = x[b].T
  - QT, KT: (512, 2048) = heads-major (8*64 rows), computed as W.T-chunks
    (stationary) x xT (moving)
  - Vhat: (2048, 520) = per head [V_h (64 cols) | 1.0], the ones column comes
    from a zero weight column + bias 1.0; it makes the PV matmul emit the
    softmax denominator as row 64 of each head's output
  - scores^T tiles (s=128, q=512x2): K=64 matmuls; exp on ACT with scale=1/8
    (no max subtraction: scores ~ N(0,1), exp is fp32-safe)
  - P^T V-hat: PSUM-accumulated over 16 s-blocks -> (65, 1024) per (head, jc)
  - normalize: reciprocal of row 64, partition-broadcast via DRAM bounce,
    multiply rows 0..63 -> attnT (512, 2048)
  - out-projection: attnT chunks stationary x woT moving -> out (2048, 1024)
    partial, bias added on head-group-0 cores only.
"""

import numpy as np

B = 4
S = 2048          # SQ == SKV
D = 1024
H_PER_CORE = 8
DH = 64
DC = H_PER_CORE * DH            # 512 head-concat dims per core
DHP = DH + 1                    # V-hat column block per head (64 + ones col)
N_CORES = 8
FP32 = None                     # set at build time (mybir.dt.float32)

_CACHE = {}


def _build_program(repeat=1):
    import concourse.bass as bass
    import concourse.tile as tile
    from concourse import bacc, mybir

    f32 = mybir.dt.float32
    nc = bacc.Bacc("TRN2", target_bir_lowering=False, debug=False,
                   enable_asserts=False, num_devices=N_CORES)

    qT = nc.dram_tensor("qT", [D, S], f32, kind="ExternalInput").ap()
    kvT = nc.dram_tensor("kvT", [D, S], f32, kind="ExternalInput").ap()
    wqT = nc.dram_tensor("wqT", [D, DC], f32, kind="ExternalInput").ap()
    wkT = nc.dram_tensor("wkT", [D, DC], f32, kind="ExternalInput").ap()
    wvh = nc.dram_tensor("wvh", [D, H_PER_CORE * DHP], f32, kind="ExternalInput").ap()
    bq = nc.dram_tensor("bq", [DC], f32, kind="ExternalInput").ap()
    bk = nc.dram_tensor("bk", [DC], f32, kind="ExternalInput").ap()
    bvh = nc.dram_tensor("bvh", [H_PER_CORE * DHP], f32, kind="ExternalInput").ap()
    woT = nc.dram_tensor("woT", [DC, D], f32, kind="ExternalInput").ap()
    bo = nc.dram_tensor("bo", [D], f32, kind="ExternalInput").ap()
    out = nc.dram_tensor("out", [S, D], f32, kind="ExternalOutput").ap()

    VW = H_PER_CORE * DHP       # 520
    KC = D // 128               # 8 contraction chunks for projections
    NM = DC // 128              # 4 partition chunks of QT/KT

    with tile.TileContext(nc) as tc:
      def _emit():
        # ---- persistent SBUF tensors --------------------------------------
        with tc.tile_pool(name="persist", bufs=1) as persist:
            qt_t = [persist.tile([128, S], f32, tag=f"qt{m}", name=f"qt{m}") for m in range(NM)]
            kt_t = [persist.tile([128, S], f32, tag=f"kt{m}", name=f"kt{m}") for m in range(NM)]
            vh_t = [persist.tile([128, VW], f32, tag=f"vh{sb}", name=f"vh{sb}") for sb in range(S // 128)]

            # biases: bq/bk as (128, NM) per-partition scalars; bvh broadcast
            bq_t = persist.tile([128, NM], f32, tag="bq")
            bk_t = persist.tile([128, NM], f32, tag="bk")
            bvh_t = persist.tile([128, VW], f32, tag="bvh")
            bo_t = persist.tile([128, D], f32, tag="bo")

            def col_ap(vec, n):  # (n*128,) dram vector -> (128, n) column tile ap
                return bass.AP(tensor=vec.tensor, offset=vec.offset,
                               ap=[[1, 128], [128, n]])

            def bcast_ap(vec, p, w):  # (w,) dram vector -> (p, w) broadcast
                return bass.AP(tensor=vec.tensor, offset=vec.offset,
                               ap=[[0, p], [1, w]])

            nc.sync.dma_start(out=bq_t, in_=col_ap(bq, NM))
            nc.sync.dma_start(out=bk_t, in_=col_ap(bk, NM))
            nc.sync.dma_start(out=bvh_t, in_=bcast_ap(bvh, 128, VW))
            nc.sync.dma_start(out=bo_t, in_=bcast_ap(bo, 128, D))

            # ---- phase 1: projections, in 4 passes over s-quarters --------
            SQW = 512                       # s-quarter width
            with tc.tile_pool(name="wpool", bufs=1) as wpool:
                wq_t = [wpool.tile([128, DC], f32, tag=f"wq{k}", name=f"wq{k}") for k in range(KC)]
                wk_t = [wpool.tile([128, DC], f32, tag=f"wk{k}", name=f"wk{k}") for k in range(KC)]
                wv_t = [wpool.tile([128, VW], f32, tag=f"wv{k}", name=f"wv{k}") for k in range(KC)]
                for k in range(KC):
                    nc.sync.dma_start(out=wq_t[k], in_=wqT[k * 128:(k + 1) * 128, :])
                    nc.sync.dma_start(out=wk_t[k], in_=wkT[k * 128:(k + 1) * 128, :])
                    nc.sync.dma_start(out=wv_t[k], in_=wvh[k * 128:(k + 1) * 128, :])

                with tc.tile_pool(name="xq", bufs=1) as xq, \
                     tc.tile_pool(name="xkv", bufs=1) as xkv, \
                     tc.tile_pool(name="pp", bufs=2, space="PSUM") as pp, \
                     tc.tile_pool(name="ppv", bufs=2, space="PSUM") as ppv:
                    for sq in range(S // SQW):
                        ssl = slice(sq * SQW, (sq + 1) * SQW)
                        q_c = [xq.tile([128, SQW], f32, tag=f"q{k}", name=f"q{k}") for k in range(KC)]
                        kv_c = [xkv.tile([128, SQW], f32, tag=f"kv{k}", name=f"kv{k}") for k in range(KC)]
                        for k in range(KC):
                            nc.sync.dma_start(out=q_c[k], in_=qT[k * 128:(k + 1) * 128, ssl])
                            nc.sync.dma_start(out=kv_c[k], in_=kvT[k * 128:(k + 1) * 128, ssl])

                        for m in range(NM):
                            msl = slice(m * 128, (m + 1) * 128)
                            ps = pp.tile([128, SQW], f32, tag="proj")
                            for k in range(KC):
                                nc.tensor.matmul(ps, wq_t[k][:, msl], q_c[k],
                                                 start=(k == 0), stop=(k == KC - 1))
                            nc.vector.tensor_scalar_add(qt_t[m][:, ssl], ps, bq_t[:, m:m + 1])
                        for m in range(NM):
                            msl = slice(m * 128, (m + 1) * 128)
                            ps = pp.tile([128, SQW], f32, tag="proj")
                            for k in range(KC):
                                nc.tensor.matmul(ps, wk_t[k][:, msl], kv_c[k],
                                                 start=(k == 0), stop=(k == KC - 1))
                            nc.vector.tensor_scalar_add(kt_t[m][:, ssl], ps, bk_t[:, m:m + 1])
                        for sm in range(SQW // 128):
                            sb = sq * (SQW // 128) + sm
                            smsl = slice(sm * 128, (sm + 1) * 128)
                            psv = ppv.tile([128, 1024], f32, tag="vproj")
                            for k in range(KC):
                                nc.tensor.matmul(psv[:, 0:512], kv_c[k][:, smsl], wv_t[k][:, 0:512],
                                                 start=(k == 0), stop=(k == KC - 1))
                                nc.tensor.matmul(psv[:, 512:VW], kv_c[k][:, smsl], wv_t[k][:, 512:VW],
                                                 start=(k == 0), stop=(k == KC - 1))
                            nc.vector.tensor_add(vh_t[sb], psv[:, 0:VW], bvh_t)

            # ---- phase 2: attention per (head, q-chunk of 1024) -----------
            # attnT tiles live in their own pool so their SBUF space only
            # exists after the phase-1 weight pool is released
            attn_cm = tc.tile_pool(name="attn", bufs=1)
            attn_pool = attn_cm.__enter__()
            at_t = [attn_pool.tile([128, S], f32, tag=f"at{m}", name=f"at{m}")
                    for m in range(NM)]
            JW = 1024
            with tc.tile_pool(name="sps", bufs=2, space="PSUM") as sps, \
                 tc.tile_pool(name="pvs", bufs=2, space="PSUM") as pvs, \
                 tc.tile_pool(name="pt", bufs=3) as ptp, \
                 tc.tile_pool(name="nrm", bufs=3) as nrm, \
                 tc.tile_pool(name="dscr", bufs=3, space="DRAM") as dscr:
                for h in range(H_PER_CORE):
                    ht = h // 2
                    hsl = slice((h % 2) * 64, (h % 2) * 64 + 64)
                    vsl = slice(h * DHP, (h + 1) * DHP)
                    for jc in range(S // JW):
                        jsl = slice(jc * JW, (jc + 1) * JW)
                        pv = pvs.tile([DHP, JW], f32, tag="pv")
                        for sb in range(S // 128):
                            sbsl = slice(sb * 128, (sb + 1) * 128)
                            sp = sps.tile([128, JW], f32, tag="sc")
                            for n in range(JW // 512):
                                nc.tensor.matmul(
                                    sp[:, n * 512:(n + 1) * 512],
                                    kt_t[ht][hsl, sbsl],
                                    qt_t[ht][hsl, jc * JW + n * 512: jc * JW + (n + 1) * 512],
                                    start=True, stop=True)
                            p_t = ptp.tile([128, JW], f32, tag="p")
                            nc.scalar.activation(p_t, sp, mybir.ActivationFunctionType.Exp,
                                                 scale=0.125)
                            for n in range(JW // 512):
                                nsl = slice(n * 512, (n + 1) * 512)
                                nc.tensor.matmul(pv[:, nsl], vh_t[sb][:, vsl], p_t[:, nsl],
                                                 start=(sb == 0), stop=(sb == S // 128 - 1))
                        # normalize rows 0..63 by reciprocal of row 64
                        rec = nrm.tile([1, JW], f32, tag="rec")
                        nc.vector.reciprocal(rec, pv[64:65, :])
                        scr = dscr.tile([1, JW], f32, tag="scr")
                        nc.sync.dma_start(out=scr, in_=rec)
                        recb = nrm.tile([64, JW], f32, tag="recb")
                        sc = scr[0, :]
                        nc.sync.dma_start(
                            out=recb,
                            in_=bass.AP(tensor=sc.tensor, offset=sc.offset,
                                        ap=[[0, 64]] + sc.ap))
                        nc.vector.tensor_mul(at_t[ht][hsl, jsl], pv[0:64, :], recb)

            # ---- phase 3: partial out-projection --------------------------
            with tc.tile_pool(name="wo", bufs=1) as wop, \
                 tc.tile_pool(name="ops", bufs=2, space="PSUM") as ops, \
                 tc.tile_pool(name="ot", bufs=3) as otp:
                wo_t = [wop.tile([128, D], f32, tag=f"wo{k}", name=f"wo{k}") for k in range(NM)]
                for k in range(NM):
                    nc.sync.dma_start(out=wo_t[k], in_=woT[k * 128:(k + 1) * 128, :])
                for qm in range(S // 128):
                    qsl = slice(qm * 128, (qm + 1) * 128)
                    for n in range(D // 512):
                        nsl = slice(n * 512, (n + 1) * 512)
                        po = ops.tile([128, 512], f32, tag="po")
                        for k in range(NM):
                            nc.tensor.matmul(po, at_t[k][:, qsl], wo_t[k][:, nsl],
                                             start=(k == 0), stop=(k == NM - 1))
                        o_t = otp.tile([128, 512], f32, tag="o")
                        nc.vector.tensor_add(o_t, po, bo_t[:, nsl])
                        nc.sync.dma_start(out=out[qsl, nsl], in_=o_t)
            attn_cm.__exit__(None, None, None)

      if repeat > 1:
          with tc.For_i(0, repeat, 1):
              _emit()
      else:
          _emit()

    nc.compile()
    return nc


def _get_runner(repeat=1):
    """Build the program once and return a cached jitted SPMD runner."""
    key = ("runner", repeat)
    if key in _CACHE:
        return _CACHE[key]

    import jax
    import jax.numpy as jnp
    from jax.sharding import Mesh, PartitionSpec
    from jax.experimental.shard_map import shard_map
    from concourse import mybir
    from concourse.bass2jax import (_bass_exec_p, install_neuronx_cc_hook,
                                    partition_id_tensor)

    nc = _build_program(repeat)
    install_neuronx_cc_hook()

    partition_name = nc.partition_id_tensor.name if nc.partition_id_tensor else None
    in_names, out_names, out_avals, zero_shapes = [], [], [], []
    for alloc in nc.m.functions[0].allocations:
        if not isinstance(alloc, mybir.MemoryLocationSet):
            continue
        name = alloc.memorylocations[0].name
        if alloc.kind == "ExternalInput":
            if name != partition_name:
                in_names.append(name)
        elif alloc.kind == "ExternalOutput":
            out_names.append(name)
            shape = tuple(alloc.tensor_shape)
            dtype = mybir.dt.np(alloc.dtype)
            out_avals.append(jax.core.ShapedArray(shape, dtype))
            zero_shapes.append((shape, dtype))
    n_params = len(in_names)
    n_outs = len(out_avals)
    all_in_names = list(in_names) + list(out_names)
    if partition_name is not None:
        all_in_names.append(partition_name)
    donate = tuple(range(n_params, n_params + n_outs))

    def _body(*args):
        operands = list(args)
        if partition_name is not None:
            operands.append(partition_id_tensor())
        outs = _bass_exec_p.bind(
            *operands,
            out_avals=tuple(out_avals),
            in_names=tuple(all_in_names),
            out_names=tuple(out_names),
            lowering_input_output_aliases=(),
            sim_require_finite=True,
            sim_require_nnan=True,
            nc=nc,
        )
        return tuple(outs)

    devices = jax.devices()[:N_CORES]
    mesh = Mesh(np.asarray(devices), ("core",))
    in_specs = (PartitionSpec("core"),) * (n_params + n_outs)
    out_specs = (PartitionSpec("core"),) * n_outs
    sharded = jax.jit(
        shard_map(_body, mesh=mesh, in_specs=in_specs, out_specs=out_specs,
                  check_rep=False),
        donate_argnums=donate, keep_unused=True)

    def run(in_maps):
        concat_in = [np.concatenate([np.asarray(m[name]) for m in in_maps], axis=0)
                     for name in in_names]
        concat_zeros = [np.zeros((N_CORES * s[0], *s[1:]), d) for s, d in zero_shapes]
        out_arrs = sharded(*concat_in, *concat_zeros)
        out_arrs = [np.asarray(a) for a in jax.block_until_ready(out_arrs)]
        return [
            {name: out_arrs[i].reshape(N_CORES, *out_avals[i].shape)[c]
             for i, name in enumerate(out_names)}
            for c in range(N_CORES)
        ]

    _CACHE[("internals", repeat)] = {
        "sharded": sharded, "mesh": mesh, "in_names": in_names,
        "out_names": out_names, "zero_shapes": zero_shapes, "nc": nc,
    }
    _CACHE[key] = run
    return run


def _prep_in_maps(query, key_value, Wq, bq, Wk, bk, Wv, bv, Wo, bo):
    f = np.float32
    in_maps = []
    for c in range(N_CORES):
        b, hg = c // 2, c % 2
        sl = slice(hg * DC, (hg + 1) * DC)
        wv_s = np.asarray(Wv, f)[sl, :].T.reshape(D, H_PER_CORE, DH)
        wvh = np.concatenate([wv_s, np.zeros((D, H_PER_CORE, 1), f)], axis=2)
        bv_s = np.asarray(bv, f)[sl].reshape(H_PER_CORE, DH)
        bvh = np.concatenate([bv_s, np.ones((H_PER_CORE, 1), f)], axis=1)
        in_maps.append({
            "qT": np.ascontiguousarray(np.asarray(query, f)[b].T),
            "kvT": np.ascontiguousarray(np.asarray(key_value, f)[b].T),
            "wqT": np.ascontiguousarray(np.asarray(Wq, f)[sl, :].T),
            "wkT": np.ascontiguousarray(np.asarray(Wk, f)[sl, :].T),
            "wvh": np.ascontiguousarray(wvh.reshape(D, H_PER_CORE * DHP)),
            "bq": np.ascontiguousarray(np.asarray(bq, f)[sl]),
            "bk": np.ascontiguousarray(np.asarray(bk, f)[sl]),
            "bvh": np.ascontiguousarray(bvh.reshape(H_PER_CORE * DHP)),
            "woT": np.ascontiguousarray(np.asarray(Wo, f)[:, sl].T),
            "bo": (np.asarray(bo, f) if hg == 0 else np.zeros(D, f)),
        })
    return in_maps


def kernel(query, key_value, Wq, bq, Wk, bk, Wv, bv, Wo, bo):
    run = _get_runner()
    in_maps = _prep_in_maps(query, key_value, Wq, bq, Wk, bk, Wv, bv, Wo, bo)
    results = run(in_maps)
    out = np.empty((B, S, D), np.float32)
    for b in range(B):
        out[b] = results[2 * b]["out"] + results[2 * b + 1]["out"]
    return out


# revision 10
# speedup vs baseline: 100.2026x; 3.1134x over previous
"""Cross-attention kernel for 8 TRN2 NeuronCores (Bass/Tile, SPMD).

Problem: B=4, SQ=SKV=2048, D_MODEL=1024, H=16 heads, Dh=64, fp32.
    Q = q @ Wq.T + bq; K = kv @ Wk.T + bk; V = kv @ Wv.T + bv
    out = softmax(Q K^T / sqrt(Dh)) V  -> concat heads -> @ Wo.T + bo

Sharding: 8 cores = 4 batches x 2 head-groups (8 heads each). Each core
computes its batch's projections for its 8 heads, full attention for those
heads, and a partial out-projection (its 512 columns of the head-concat dim).
The host sums the two partials per batch (no device collectives needed).

Device layout (everything transposed so matmul contractions sit on the
partition axis):
  - inputs qT/kvT: (1024, 2048) = x[b].T
  - QT, KT: (512, 2048) = heads-major (8*64 rows), computed as W.T-chunks
    (stationary) x xT (moving)
  - Vhat: (2048, 520) = per head [V_h (64 cols) | 1.0], the ones column comes
    from a zero weight column + bias 1.0; it makes the PV matmul emit the
    softmax denominator as row 64 of each head's output
  - scores^T tiles (s=128, q=512x2): K=64 matmuls; exp on ACT with scale=1/8
    (no max subtraction: scores ~ N(0,1), exp is fp32-safe)
  - P^T V-hat: PSUM-accumulated over 16 s-blocks -> (65, 1024) per (head, jc)
  - normalize: reciprocal of row 64, partition-broadcast via DRAM bounce,
    multiply rows 0..63 -> attnT (512, 2048)
  - out-projection: attnT chunks stationary x woT moving -> out (2048, 1024)
    partial, bias added on head-group-0 cores only.
"""

import numpy as np

B = 4
S = 2048          # SQ == SKV
D = 1024
H_PER_CORE = 8
DH = 64
DC = H_PER_CORE * DH            # 512 head-concat dims per core
DHP = DH + 1                    # V-hat column block per head (64 + ones col)
N_CORES = 8
FP32 = None                     # set at build time (mybir.dt.float32)

_CACHE = {}


def _build_program(repeat=1):
    import concourse.bass as bass
    import concourse.tile as tile
    from concourse import bacc, mybir

    f32 = mybir.dt.float32
    f32r = mybir.dt.float32r
    nc = bacc.Bacc("TRN2", target_bir_lowering=False, debug=False,
                   enable_asserts=False, num_devices=N_CORES)

    qT = nc.dram_tensor("qT", [D, S], f32r, kind="ExternalInput").ap()
    kvT = nc.dram_tensor("kvT", [D, S], f32r, kind="ExternalInput").ap()
    wqT = nc.dram_tensor("wqT", [D, DC], f32r, kind="ExternalInput").ap()
    wkT = nc.dram_tensor("wkT", [D, DC], f32r, kind="ExternalInput").ap()
    wvh = nc.dram_tensor("wvh", [D, H_PER_CORE * DHP], f32r, kind="ExternalInput").ap()
    bq = nc.dram_tensor("bq", [DC], f32, kind="ExternalInput").ap()
    bk = nc.dram_tensor("bk", [DC], f32, kind="ExternalInput").ap()
    bvh = nc.dram_tensor("bvh", [H_PER_CORE * DHP], f32, kind="ExternalInput").ap()
    woT = nc.dram_tensor("woT", [DC, D], f32r, kind="ExternalInput").ap()
    bo = nc.dram_tensor("bo", [D], f32, kind="ExternalInput").ap()
    out = nc.dram_tensor("out", [S, D], f32, kind="ExternalOutput").ap()

    VW = H_PER_CORE * DHP       # 520
    KC = D // 128               # 8 contraction chunks for projections
    NM = DC // 128              # 4 partition chunks of QT/KT

    with tile.TileContext(nc) as tc:
      def _emit():
        # ---- persistent SBUF tensors --------------------------------------
        with tc.tile_pool(name="persist", bufs=1) as persist:
            qt_t = [persist.tile([128, S], f32r, tag=f"qt{m}", name=f"qt{m}") for m in range(NM)]
            kt_t = [persist.tile([128, S], f32r, tag=f"kt{m}", name=f"kt{m}") for m in range(NM)]
            vh_t = [persist.tile([128, VW], f32r, tag=f"vh{sb}", name=f"vh{sb}") for sb in range(S // 128)]

            # biases: bq/bk as (128, NM) per-partition scalars; bvh broadcast
            bq_t = persist.tile([128, NM], f32, tag="bq")
            bk_t = persist.tile([128, NM], f32, tag="bk")
            bvh_t = persist.tile([128, VW], f32, tag="bvh")
            bo_t = persist.tile([128, D], f32, tag="bo")

            def col_ap(vec, n):  # (n*128,) dram vector -> (128, n) column tile ap
                return bass.AP(tensor=vec.tensor, offset=vec.offset,
                               ap=[[1, 128], [128, n]])

            def bcast_ap(vec, p, w):  # (w,) dram vector -> (p, w) broadcast
                return bass.AP(tensor=vec.tensor, offset=vec.offset,
                               ap=[[0, p], [1, w]])

            nc.sync.dma_start(out=bq_t, in_=col_ap(bq, NM))
            nc.sync.dma_start(out=bk_t, in_=col_ap(bk, NM))
            nc.sync.dma_start(out=bvh_t, in_=bcast_ap(bvh, 128, VW))
            nc.sync.dma_start(out=bo_t, in_=bcast_ap(bo, 128, D))

            # ---- phase 1: projections, in 4 passes over s-quarters --------
            SQW = 512                       # s-quarter width
            with tc.tile_pool(name="wpool", bufs=1) as wpool:
                wq_t = [wpool.tile([128, DC], f32r, tag=f"wq{k}", name=f"wq{k}") for k in range(KC)]
                wk_t = [wpool.tile([128, DC], f32r, tag=f"wk{k}", name=f"wk{k}") for k in range(KC)]
                wv_t = [wpool.tile([128, VW], f32r, tag=f"wv{k}", name=f"wv{k}") for k in range(KC)]
                for k in range(KC):
                    nc.sync.dma_start(out=wq_t[k], in_=wqT[k * 128:(k + 1) * 128, :])
                    nc.sync.dma_start(out=wk_t[k], in_=wkT[k * 128:(k + 1) * 128, :])
                    nc.sync.dma_start(out=wv_t[k], in_=wvh[k * 128:(k + 1) * 128, :])

                with tc.tile_pool(name="xq", bufs=1) as xq, \
                     tc.tile_pool(name="xkv", bufs=1) as xkv, \
                     tc.tile_pool(name="pp", bufs=2, space="PSUM") as pp, \
                     tc.tile_pool(name="ppv", bufs=2, space="PSUM") as ppv:
                    for sq in range(S // SQW):
                        ssl = slice(sq * SQW, (sq + 1) * SQW)
                        q_c = [xq.tile([128, SQW], f32r, tag=f"q{k}", name=f"q{k}") for k in range(KC)]
                        kv_c = [xkv.tile([128, SQW], f32r, tag=f"kv{k}", name=f"kv{k}") for k in range(KC)]
                        for k in range(KC):
                            nc.sync.dma_start(out=q_c[k], in_=qT[k * 128:(k + 1) * 128, ssl])
                            nc.sync.dma_start(out=kv_c[k], in_=kvT[k * 128:(k + 1) * 128, ssl])

                        for m in range(NM):
                            msl = slice(m * 128, (m + 1) * 128)
                            ps = pp.tile([128, SQW], f32, tag="proj")
                            for k in range(KC):
                                nc.tensor.matmul(ps, wq_t[k][:, msl], q_c[k],
                                                 start=(k == 0), stop=(k == KC - 1))
                            nc.vector.tensor_scalar_add(qt_t[m][:, ssl], ps, bq_t[:, m:m + 1])
                        for m in range(NM):
                            msl = slice(m * 128, (m + 1) * 128)
                            ps = pp.tile([128, SQW], f32, tag="proj")
                            for k in range(KC):
                                nc.tensor.matmul(ps, wk_t[k][:, msl], kv_c[k],
                                                 start=(k == 0), stop=(k == KC - 1))
                            nc.vector.tensor_scalar_add(kt_t[m][:, ssl], ps, bk_t[:, m:m + 1])
                        for sm in range(SQW // 128):
                            sb = sq * (SQW // 128) + sm
                            smsl = slice(sm * 128, (sm + 1) * 128)
                            psv = ppv.tile([128, 1024], f32, tag="vproj")
                            for k in range(KC):
                                nc.tensor.matmul(psv[:, 0:512], kv_c[k][:, smsl], wv_t[k][:, 0:512],
                                                 start=(k == 0), stop=(k == KC - 1))
                                nc.tensor.matmul(psv[:, 512:VW], kv_c[k][:, smsl], wv_t[k][:, 512:VW],
                                                 start=(k == 0), stop=(k == KC - 1))
                            nc.vector.tensor_add(vh_t[sb], psv[:, 0:VW], bvh_t)

            # ---- phase 2: attention per (head, q-chunk of 1024) -----------
            # attnT tiles live in their own pool so their SBUF space only
            # exists after the phase-1 weight pool is released
            attn_cm = tc.tile_pool(name="attn", bufs=1)
            attn_pool = attn_cm.__enter__()
            at_t = [attn_pool.tile([128, S], f32r, tag=f"at{m}", name=f"at{m}")
                    for m in range(NM)]
            JW = 1024
            with tc.tile_pool(name="sps", bufs=2, space="PSUM") as sps, \
                 tc.tile_pool(name="pvs", bufs=2, space="PSUM") as pvs, \
                 tc.tile_pool(name="pt", bufs=3) as ptp, \
                 tc.tile_pool(name="nrm", bufs=3) as nrm, \
                 tc.tile_pool(name="dscr", bufs=3, space="DRAM") as dscr:
                for h in range(H_PER_CORE):
                    ht = h // 2
                    hsl = slice((h % 2) * 64, (h % 2) * 64 + 64)
                    vsl = slice(h * DHP, (h + 1) * DHP)
                    for jc in range(S // JW):
                        jsl = slice(jc * JW, (jc + 1) * JW)
                        pv = pvs.tile([DHP, JW], f32, tag="pv")
                        for sb in range(S // 128):
                            sbsl = slice(sb * 128, (sb + 1) * 128)
                            sp = sps.tile([128, JW], f32, tag="sc")
                            for n in range(JW // 512):
                                nc.tensor.matmul(
                                    sp[:, n * 512:(n + 1) * 512],
                                    kt_t[ht][hsl, sbsl],
                                    qt_t[ht][hsl, jc * JW + n * 512: jc * JW + (n + 1) * 512],
                                    start=True, stop=True)
                            p_t = ptp.tile([128, JW], f32r, tag="p")
                            nc.scalar.activation(p_t, sp, mybir.ActivationFunctionType.Exp,
                                                 scale=0.125)
                            for n in range(JW // 512):
                                nsl = slice(n * 512, (n + 1) * 512)
                                nc.tensor.matmul(pv[:, nsl], vh_t[sb][:, vsl], p_t[:, nsl],
                                                 start=(sb == 0), stop=(sb == S // 128 - 1))
                        # normalize rows 0..63 by reciprocal of row 64
                        rec = nrm.tile([1, JW], f32, tag="rec")
                        nc.vector.reciprocal(rec, pv[64:65, :])
                        scr = dscr.tile([1, JW], f32, tag="scr")
                        nc.sync.dma_start(out=scr, in_=rec)
                        recb = nrm.tile([64, JW], f32, tag="recb")
                        sc = scr[0, :]
                        nc.sync.dma_start(
                            out=recb,
                            in_=bass.AP(tensor=sc.tensor, offset=sc.offset,
                                        ap=[[0, 64]] + sc.ap))
                        nc.vector.tensor_mul(at_t[ht][hsl, jsl], pv[0:64, :], recb)

            # ---- phase 3: partial out-projection --------------------------
            with tc.tile_pool(name="wo", bufs=1) as wop, \
                 tc.tile_pool(name="ops", bufs=2, space="PSUM") as ops, \
                 tc.tile_pool(name="ot", bufs=3) as otp:
                wo_t = [wop.tile([128, D], f32r, tag=f"wo{k}", name=f"wo{k}") for k in range(NM)]
                for k in range(NM):
                    nc.sync.dma_start(out=wo_t[k], in_=woT[k * 128:(k + 1) * 128, :])
                for qm in range(S // 128):
                    qsl = slice(qm * 128, (qm + 1) * 128)
                    for n in range(D // 512):
                        nsl = slice(n * 512, (n + 1) * 512)
                        po = ops.tile([128, 512], f32, tag="po")
                        for k in range(NM):
                            nc.tensor.matmul(po, at_t[k][:, qsl], wo_t[k][:, nsl],
                                             start=(k == 0), stop=(k == NM - 1))
                        o_t = otp.tile([128, 512], f32, tag="o")
                        nc.vector.tensor_add(o_t, po, bo_t[:, nsl])
                        nc.sync.dma_start(out=out[qsl, nsl], in_=o_t)
            attn_cm.__exit__(None, None, None)

      if repeat > 1:
          with tc.For_i(0, repeat, 1):
              _emit()
      else:
          _emit()

    nc.compile()
    return nc


def _get_runner(repeat=1):
    """Build the program once and return a cached jitted SPMD runner."""
    key = ("runner", repeat)
    if key in _CACHE:
        return _CACHE[key]

    import jax
    import jax.numpy as jnp
    from jax.sharding import Mesh, PartitionSpec
    from jax.experimental.shard_map import shard_map
    from concourse import mybir
    from concourse.bass2jax import (_bass_exec_p, install_neuronx_cc_hook,
                                    partition_id_tensor)

    nc = _build_program(repeat)
    install_neuronx_cc_hook()

    partition_name = nc.partition_id_tensor.name if nc.partition_id_tensor else None
    in_names, out_names, out_avals, zero_shapes = [], [], [], []
    for alloc in nc.m.functions[0].allocations:
        if not isinstance(alloc, mybir.MemoryLocationSet):
            continue
        name = alloc.memorylocations[0].name
        if alloc.kind == "ExternalInput":
            if name != partition_name:
                in_names.append(name)
        elif alloc.kind == "ExternalOutput":
            out_names.append(name)
            shape = tuple(alloc.tensor_shape)
            dtype = mybir.dt.np(alloc.dtype)
            out_avals.append(jax.core.ShapedArray(shape, dtype))
            zero_shapes.append((shape, dtype))
    n_params = len(in_names)
    n_outs = len(out_avals)
    all_in_names = list(in_names) + list(out_names)
    if partition_name is not None:
        all_in_names.append(partition_name)
    donate = tuple(range(n_params, n_params + n_outs))

    def _body(*args):
        operands = list(args)
        if partition_name is not None:
            operands.append(partition_id_tensor())
        outs = _bass_exec_p.bind(
            *operands,
            out_avals=tuple(out_avals),
            in_names=tuple(all_in_names),
            out_names=tuple(out_names),
            lowering_input_output_aliases=(),
            sim_require_finite=True,
            sim_require_nnan=True,
            nc=nc,
        )
        return tuple(outs)

    devices = jax.devices()[:N_CORES]
    mesh = Mesh(np.asarray(devices), ("core",))
    in_specs = (PartitionSpec("core"),) * (n_params + n_outs)
    out_specs = (PartitionSpec("core"),) * n_outs
    sharded = jax.jit(
        shard_map(_body, mesh=mesh, in_specs=in_specs, out_specs=out_specs,
                  check_rep=False),
        donate_argnums=donate, keep_unused=True)

    def run(in_maps):
        concat_in = [np.concatenate([np.asarray(m[name]) for m in in_maps], axis=0)
                     for name in in_names]
        concat_zeros = [np.zeros((N_CORES * s[0], *s[1:]), d) for s, d in zero_shapes]
        out_arrs = sharded(*concat_in, *concat_zeros)
        out_arrs = [np.asarray(a) for a in jax.block_until_ready(out_arrs)]
        return [
            {name: out_arrs[i].reshape(N_CORES, *out_avals[i].shape)[c]
             for i, name in enumerate(out_names)}
            for c in range(N_CORES)
        ]

    _CACHE[("internals", repeat)] = {
        "sharded": sharded, "mesh": mesh, "in_names": in_names,
        "out_names": out_names, "zero_shapes": zero_shapes, "nc": nc,
    }
    _CACHE[key] = run
    return run


def _prep_in_maps(query, key_value, Wq, bq, Wk, bk, Wv, bv, Wo, bo):
    f = np.float32
    in_maps = []
    for c in range(N_CORES):
        b, hg = c // 2, c % 2
        sl = slice(hg * DC, (hg + 1) * DC)
        wv_s = np.asarray(Wv, f)[sl, :].T.reshape(D, H_PER_CORE, DH)
        wvh = np.concatenate([wv_s, np.zeros((D, H_PER_CORE, 1), f)], axis=2)
        bv_s = np.asarray(bv, f)[sl].reshape(H_PER_CORE, DH)
        bvh = np.concatenate([bv_s, np.ones((H_PER_CORE, 1), f)], axis=1)
        in_maps.append({
            "qT": np.ascontiguousarray(np.asarray(query, f)[b].T),
            "kvT": np.ascontiguousarray(np.asarray(key_value, f)[b].T),
            "wqT": np.ascontiguousarray(np.asarray(Wq, f)[sl, :].T),
            "wkT": np.ascontiguousarray(np.asarray(Wk, f)[sl, :].T),
            "wvh": np.ascontiguousarray(wvh.reshape(D, H_PER_CORE * DHP)),
            "bq": np.ascontiguousarray(np.asarray(bq, f)[sl]),
            "bk": np.ascontiguousarray(np.asarray(bk, f)[sl]),
            "bvh": np.ascontiguousarray(bvh.reshape(H_PER_CORE * DHP)),
            "woT": np.ascontiguousarray(np.asarray(Wo, f)[:, sl].T),
            "bo": (np.asarray(bo, f) if hg == 0 else np.zeros(D, f)),
        })
    return in_maps


def kernel(query, key_value, Wq, bq, Wk, bk, Wv, bv, Wo, bo):
    run = _get_runner()
    in_maps = _prep_in_maps(query, key_value, Wq, bq, Wk, bk, Wv, bv, Wo, bo)
    results = run(in_maps)
    out = np.empty((B, S, D), np.float32)
    for b in range(B):
        out[b] = results[2 * b]["out"] + results[2 * b + 1]["out"]
    return out
